# revision 22
# baseline (speedup 1.0000x reference)
"""Trainium2 Bass kernel for KL-divergence 1-NN label lookup (AnchorStore).

reference:
    self[k]  = mean_d a[k,d]*log a[k,d]
    cross    = einsum('kd,bd->kb', a, log q) / D
    kl[b,k]  = self[k] - cross[k,b]
    out[b]   = queue_label[argmin_k kl[b,k]]

Strategy (8 NeuronCores, D-sharded):
    Each core owns a D-slice (padded with 1.0 so log()=0 contributes
    nothing). Per core, compute in SUM units (scale-invariant for argmin):
        m_partial[b,k] = sum_d lq[d,b]*at[d,k] - sum_d at[d,k]*log(at[d,k])
    via TensorE: stationary lq tiles [128d,128b] x moving at [128d,512k]
    accumulated in PSUM; the -self term comes from a (-1)-matrix stationary
    times t = at*log(at). A ReduceScatter(add) gives each core the fully
    summed m for its 32-query slice; local argmax + label mask produce 32
    int32 labels per core, concatenated on the host.
"""

import os
import sys

import numpy as np

sys.path.insert(0, "/opt/trn_rl_repo")

from concourse import bacc, bass, mybir, tile  # noqa: E402
from concourse import bass_utils  # noqa: E402

K = 2048
B = 256
D = 50257
NCORES = 8
DSH = 6400  # padded per-core D-slice (50 tiles of 128)
BS = B // NCORES
F32 = mybir.dt.float32


def build(mm_dtype=F32, dsh=DSH, debug_out=False, passes=1):
    """Build the SPMD Bass graph for one core (all cores identical).

    passes=1: single dense d-loop over full-width at tiles, one RS at end.
    passes=2: two k-half passes; pass 0's RS overlaps pass 1's compute, and
              the -self accumulator is shared between b-tiles (fewer MMs).
    """
    nt = dsh // 128          # d-tiles per core
    kc = K // 512            # k chunks (psum banks per b-tile)
    cpp = kc // passes       # chunks per pass
    kw = K // passes         # k columns per pass
    nc = bacc.Bacc(
        "TRN2", target_bir_lowering=False, debug=False, num_devices=NCORES
    )
    # Matmul-facing tensors carry mm_dtype (float32r requires typed
    # producers so walrus sees rounded operands); all else stays f32.
    at_d = nc.dram_tensor("at", [dsh, K], mm_dtype, kind="ExternalInput")
    qt_d = nc.dram_tensor("qt", [dsh, B], F32, kind="ExternalInput")
    lab_d = nc.dram_tensor("lab1", [BS, K], F32, kind="ExternalInput")
    out_d = nc.dram_tensor("out", [BS], mybir.dt.int32, kind="ExternalOutput")
    if debug_out:
        mdbg_d = nc.dram_tensor("mdbg", [BS, K], F32, kind="ExternalOutput")

    LN = mybir.ActivationFunctionType.Ln
    AX = mybir.AxisListType.X
    OP = mybir.AluOpType

    with tile.TileContext(nc) as tc:
        with (
            tc.tile_pool(name="const", bufs=1) as constp,
            tc.tile_pool(name="lqp", bufs=1) as lqp,
            tc.tile_pool(name="qinp", bufs=4) as qinp,
            tc.tile_pool(name="atp", bufs=4) as atp,
            tc.tile_pool(name="latp", bufs=2) as latp,
            tc.tile_pool(name="tpp", bufs=2) as tpp,
            tc.tile_pool(name="msbp", bufs=2) as msbp,
            tc.tile_pool(name="epp", bufs=2) as epp,
            tc.tile_pool(name="psp", bufs=1, space="PSUM") as psp,
            tc.tile_pool(name="dramp", bufs=1, space="DRAM") as dramp,
        ):
            negones = constp.tile([128, 128], mm_dtype)
            if mm_dtype == F32:
                nc.gpsimd.memset(negones[:], -1.0)
            else:
                negones_f = constp.tile([128, 128], F32)
                nc.gpsimd.memset(negones_f[:], -1.0)
                nc.vector.tensor_copy(negones[:], negones_f[:])
            lab1 = constp.tile([BS, K], F32)
            nc.sync.dma_start(lab1[:], lab_d[:])

            # lq = log(query^T), resident in SBUF: [128, nt*B]
            lq = lqp.tile([128, nt * B], mm_dtype)
            for t in range(nt):
                qtile = qinp.tile([128, B], F32)
                nc.sync.dma_start(qtile[:], qt_d[t * 128 : (t + 1) * 128, :])
                nc.scalar.activation(lq[:, t * B : (t + 1) * B], qtile[:], LN)

            # PSUM accumulators. passes=1: 8 pk banks, -self goes into pk
            # per b-tile. passes=2: 4 rotating pk banks + 2 srep banks;
            # -self shared by b-tiles, added during drain.
            pk = [
                [
                    psp.tile(
                        [128, 512],
                        F32,
                        name=f"pk_{cg}_{bt}",
                        tag=f"pk_{cg if passes == 1 else cg % cpp}_{bt}",
                    )
                    for bt in range(2)
                ]
                for cg in range(kc)
            ]
            use_srep = passes > 1
            if use_srep:
                srep = [
                    psp.tile(
                        [128, 512], F32, name=f"srep_{cg}", tag=f"srep_{cg % cpp}"
                    )
                    for cg in range(kc)
                ]

            msum = epp.tile([BS, K], F32)
            for ps in range(passes):
                k0 = ps * kw
                for t in range(nt):
                    att = atp.tile([128, kw], mm_dtype, name=f"att_{ps}_{t}",
                                   tag="att")
                    nc.sync.dma_start(
                        att[:], at_d[t * 128 : (t + 1) * 128, k0 : k0 + kw]
                    )
                    latt = latp.tile([128, kw], F32, name=f"latt_{ps}_{t}",
                                     tag="latt")
                    nc.scalar.activation(latt[:], att[:], LN)
                    tt = tpp.tile([128, kw], mm_dtype, name=f"tt_{ps}_{t}",
                                  tag="tt")
                    nc.vector.tensor_tensor(tt[:], att[:], latt[:], op=OP.mult)
                    for bt in range(2):
                        lhs = lq[:, t * B + bt * 128 : t * B + bt * 128 + 128]
                        for cl in range(cpp):
                            nc.tensor.matmul(
                                pk[ps * cpp + cl][bt][:],
                                lhs,
                                att[:, cl * 512 : (cl + 1) * 512],
                                start=(t == 0),
                                stop=(use_srep and t == nt - 1),
                            )
                    if use_srep:
                        for cl in range(cpp):
                            nc.tensor.matmul(
                                srep[ps * cpp + cl][:],
                                negones[:],
                                tt[:, cl * 512 : (cl + 1) * 512],
                                start=(t == 0),
                                stop=(t == nt - 1),
                            )
                    else:
                        for bt in range(2):
                            for cl in range(cpp):
                                nc.tensor.matmul(
                                    pk[ps * cpp + cl][bt][:],
                                    negones[:],
                                    tt[:, cl * 512 : (cl + 1) * 512],
                                    start=False,
                                    stop=(t == nt - 1),
                                )

                # Drain this pass: PSUM -> SBUF -> DRAM bounce -> RS. With
                # passes=2, pass 0's RS overlaps pass 1's compute.
                ar_in = dramp.tile(
                    [B, kw], F32, name=f"ar_in_{ps}", tag="ar_in", bufs=passes
                )
                rs_out = dramp.tile(
                    [BS, kw], F32, name=f"rs_out_{ps}", tag="rs_out",
                    bufs=passes,
                )
                if use_srep:
                    for cl in range(cpp):
                        srep_sb = msbp.tile(
                            [128, 512], F32, name=f"srep_sb_{ps}_{cl}",
                            tag="srep_sb",
                        )
                        nc.vector.tensor_copy(
                            srep_sb[:], srep[ps * cpp + cl][:]
                        )
                        for bt in range(2):
                            m_sb = msbp.tile(
                                [128, 512], F32,
                                name=f"m_sb_{ps}_{cl}_{bt}", tag=f"m_sb{bt}",
                            )
                            nc.vector.tensor_tensor(
                                m_sb[:], pk[ps * cpp + cl][bt][:], srep_sb[:],
                                op=OP.add,
                            )
                            nc.gpsimd.dma_start(
                                ar_in[
                                    bt * 128 : (bt + 1) * 128,
                                    cl * 512 : (cl + 1) * 512,
                                ],
                                m_sb[:],
                            )
                else:
                    for cl in range(cpp):
                        for bt in range(2):
                            m_sb = msbp.tile(
                                [128, 512], F32,
                                name=f"m_sb_{ps}_{cl}_{bt}", tag=f"m_sb{bt}",
                            )
                            nc.vector.tensor_copy(
                                m_sb[:], pk[ps * cpp + cl][bt][:]
                            )
                            nc.gpsimd.dma_start(
                                ar_in[
                                    bt * 128 : (bt + 1) * 128,
                                    cl * 512 : (cl + 1) * 512,
                                ],
                                m_sb[:],
                            )
                nc.gpsimd.collective_compute(
                    "ReduceScatter",
                    OP.add,
                    replica_groups=[list(range(NCORES))],
                    ins=[ar_in.opt()],
                    outs=[rs_out.opt()],
                )
                nc.sync.dma_start(msum[:, k0 : k0 + kw], rs_out[:])

            if debug_out:
                nc.sync.dma_start(mdbg_d[:], msum[:])

            # Local argmax over k + label extraction for this core's 32
            # queries; host concatenates the 8 cores' slices.
            gmax = epp.tile([BS, 1], F32)
            nc.vector.tensor_reduce(gmax[:], msum[:], axis=AX, op=OP.max)
            eq = epp.tile([BS, K], F32)
            nc.vector.tensor_scalar(
                eq[:], msum[:], gmax[:], None, op0=OP.is_equal
            )
            cand = epp.tile([BS, K], F32)
            nc.vector.tensor_tensor(cand[:], eq[:], lab1[:], op=OP.mult)
            lmax = epp.tile([BS, 1], F32)
            nc.vector.tensor_reduce(lmax[:], cand[:], axis=AX, op=OP.max)
            labf = epp.tile([BS, 1], F32)
            nc.vector.tensor_scalar_add(labf[:], lmax[:], -1.0)
            labi = epp.tile([BS, 1], mybir.dt.int32)
            nc.vector.tensor_copy(labi[:], labf[:])
            nc.sync.dma_start(out_d[:], labi[:])

    nc.compile()
    return nc


def shard_inputs(query, queue_anchor, queue_label, dsh=DSH, d_real=D):
    """Host-side layout prep: pad D with 1.0 (log 1 = 0), per-core
    transposed slices, label row replicated to the 32 epilogue rows."""
    q = np.asarray(query, np.float32)
    a = np.asarray(queue_anchor, np.float32)
    lab1 = (np.asarray(queue_label).astype(np.float32) + 1.0)[None, :]
    lab1 = np.ascontiguousarray(np.broadcast_to(lab1, (BS, lab1.shape[1])))
    in_maps = []
    for c in range(NCORES):
        lo = c * dsh
        hi = min((c + 1) * dsh, d_real)
        at = np.ones((dsh, a.shape[0]), np.float32)
        qt = np.ones((dsh, q.shape[0]), np.float32)
        if hi > lo:
            at[: hi - lo, :] = a[:, lo:hi].T
            qt[: hi - lo, :] = q[:, lo:hi].T
        in_maps.append({"at": at, "qt": qt, "lab1": lab1})
    return in_maps


_NC_CACHE = {}


def _get_nc():
    key = (
        os.environ.get("ANCHOR_MM_DTYPE", "float32r"),
        int(os.environ.get("ANCHOR_PASSES", "1")),
    )
    if key not in _NC_CACHE:
        _NC_CACHE[key] = build(
            mm_dtype=getattr(mybir.dt, key[0]), passes=key[1]
        )
    return _NC_CACHE[key]


def kernel(query, queue_anchor, queue_label):
    nc = _get_nc()
    in_maps = shard_inputs(query, queue_anchor, queue_label)
    res = bass_utils.run_bass_kernel_spmd(
        nc, in_maps, core_ids=list(range(NCORES))
    )
    out = np.concatenate(
        [np.asarray(res.results[i]["out"]) for i in range(NCORES)]
    )
    return out.astype(np.asarray(queue_label).dtype)
